# revision 59
# baseline (speedup 1.0000x reference)
"""Causal self-attention Bass/Tile kernel for 8-core TRN2.

Sharding: core c handles batch b = c//4, head-group hg = c%4 (4 heads of 16).
Each core computes a partial output y_c = attn_out_local @ W_out_slice.T of
shape (L, E) in fp16; the host sums the 4 partials per batch in fp32.

Per-core layout (L=2048, E=1024, D=64, 4 local heads):
  - QKV projection in fp32r from host-pre-transposed xT (E, L).
  - qT, kT transposed [d, l] fp16 (head pairs packed 64+64), v natural
    [l, d] fp16 with a 65th ones-column (denominator fused into AV).
  - S^T = K Q^T per head (K=64 contraction), exp on ACT (scale=1/8, no max
    subtraction: scores ~ N(0,1)), causal band masking on DVE.
  - AV in [q, d] form: stationary ex tile [kp, q], moving V [kp, 65]; PSUM
    accumulates [q, 65] per (head, q-subtile); col 64 = softmax denominator.
  - Normalize with per-partition reciprocal broadcast (DVE), transpose AO to
    [dh, l] on the PE (identity matmul), out-projection fp16 into y (L, E).
"""

import numpy as np

import concourse.bass as bass
import concourse.mybir as mybir
import concourse.tile as tile
from concourse import bacc

F32 = mybir.dt.float32
F32R = mybir.dt.float32r
F16 = mybir.dt.float16
F8 = mybir.dt.float8e4
BF16 = mybir.dt.bfloat16

B, L, H, D = 2, 2048, 16, 64
E = H * D  # 1024
HL = 4  # heads per core
DH = HL * D  # 256, local head dims
KC = E // 128  # 8 contraction chunks for projections
NQ = L // 512  # 4 q-chunks
NL = L // 128  # 16 l-tiles

# acc offsets (fp32 elements) for 6 [128, 130] pair-views in a [128, 1024]
# psum tile (2 banks of 512 els; 3 pair-views per bank, none crossing a bank
# boundary). The 8 (pair, q-subtile) groups map onto 6 slots; the last two
# reuse the earliest-completed slots.
ACC_POFF = [(k // 3) * 512 + (k % 3) * 130 for k in range(6)]


def round_fp32r(x: np.ndarray) -> np.ndarray:
    """Round fp32 to fp32r (11-bit mantissa, RNE on low 12 bits)."""
    u = np.ascontiguousarray(x, dtype=np.float32).view(np.uint32)
    lsb = (u >> 12) & np.uint32(1)
    u = u + np.uint32(0x7FF) + lsb
    u = u & np.uint32(0xFFFFF000)
    return u.view(np.float32)


def build_kernel(phases=("proj", "attn", "oproj"), reps=1, debug_out=False):
    nc = bacc.Bacc("TRN2", target_bir_lowering=False, debug=False, num_devices=8)

    xh = nc.dram_tensor("xh", [E, L], F8, kind="ExternalInput").ap()
    xl = nc.dram_tensor("xl", [E, L], F8, kind="ExternalInput").ap()
    wq3 = nc.dram_tensor("wq3", [3, E, DH], F8, kind="ExternalInput").ap()
    wk3 = nc.dram_tensor("wk3", [3, E, DH], F8, kind="ExternalInput").ap()
    wv3 = nc.dram_tensor("wv3", [3, E, DH], F8, kind="ExternalInput").ap()
    woT = nc.dram_tensor("woT", [DH, E], F16, kind="ExternalInput").ap()
    tri = nc.dram_tensor("tri", [128, 128], F16, kind="ExternalInput").ap()
    eye = nc.dram_tensor("eye", [128, 128], F16, kind="ExternalInput").ap()
    y = nc.dram_tensor("y", [L, E], F16, kind="ExternalOutput").ap()
    if debug_out:
        dQT = nc.dram_tensor("dQT", [128, 2, L], F16, kind="ExternalOutput").ap()
        dKT = nc.dram_tensor("dKT", [128, 2, L], F16, kind="ExternalOutput").ap()
        dV = nc.dram_tensor("dV", [128, NL, HL, D + 1], F16, kind="ExternalOutput").ap()
        dAO = nc.dram_tensor("dAO", [NQ, 128, 4, HL, D], F16, kind="ExternalOutput").ap()
        dAOT = nc.dram_tensor("dAOT", [128, 2, L], F16, kind="ExternalOutput").ap()

    with tile.TileContext(nc) as tc:
        with (
            tc.tile_pool(name="big", bufs=1) as big,
            tc.tile_pool(name="ao", bufs=2) as aop,
            tc.tile_pool(name="ys", bufs=4) as ysp,
            tc.tile_pool(name="rcp", bufs=8) as rcp,
            tc.tile_pool(name="exs", bufs=34) as exs,
            tc.tile_pool(name="ps_s", bufs=2, space="PSUM") as ps_s,
            tc.tile_pool(name="ps_acc", bufs=1, space="PSUM") as ps_acc,
            tc.tile_pool(name="ps_sm", bufs=2, space="PSUM") as ps_sm,
        ):
            # ---- static SBUF tensors ----
            X3 = big.tile([128, 2, KC, L], F8, tag="X3")
            WQ3 = big.tile([128, 3, KC, DH], F8, tag="WQ3")
            WK3 = big.tile([128, 3, KC, DH], F8, tag="WK3")
            WV3 = big.tile([128, 3, KC, DH], F8, tag="WV3")
            WO = big.tile([128, DH // 128, E], F16, tag="WO")
            QT = big.tile([128, HL // 2, L], F16, tag="QT")
            KT = big.tile([128, HL // 2, L], F16, tag="KT")
            V = big.tile([128, NL, HL, D + 1], F16, tag="V")
            AOT = big.tile([128, DH // 128, L], F16, tag="AOT")
            CM = big.tile([128, 128], F16, tag="CM")
            EYE = big.tile([128, 128], F16, tag="EYE")

            nc.vector.memset(V[:, :, :, D], 512.0)

            # ---- input DMA (term-0 arrays first, X by l-chunk) ----
            XARR = (xh, xl)

            def dma_x_chunk(n, t):
                sl = slice(n * 512, (n + 1) * 512)
                nc.sync.dma_start(
                    X3[:, t, :, sl],
                    XARR[t][:, sl].rearrange("(o p) l -> p o l", p=128),
                )

            def dma_w(tile_, dram, t):
                nc.sync.dma_start(
                    tile_[:, t], dram[t].rearrange("(o p) d -> p o d", p=128)
                )

            dma_w(WQ3, wq3, 0)
            dma_x_chunk(0, 0)
            dma_w(WQ3, wq3, 2)
            dma_w(WQ3, wq3, 1)
            dma_x_chunk(0, 1)
            for t in range(3):
                dma_w(WK3, wk3, t)
            for t in range(3):
                dma_w(WV3, wv3, t)
            nc.sync.dma_start(CM[:], tri)
            nc.sync.dma_start(EYE[:], eye)
            nc.sync.dma_start(WO[:], woT.rearrange("(o p) e -> p o e", p=128))
            for n in (1, 2, 3):
                for t in range(2):
                    dma_x_chunk(n, t)

            # ---- per-stage group bodies (each = one ps_sm psum tile) ----
            DR = mybir.MatmulPerfMode.DoubleRow
            # compensated terms with W' = 512*W (no denormal underflow in
            # Wlo): (Whi, xhi), (Wlo, xhi), (Whi/16, 16*xlo)
            QK_TERMS = ((0, 0), (2, 0), (1, 1))  # (w-array, x-array)
            V_TERMS = ((0, 0), (0, 2), (1, 1))  # (x-array, w-array)

            def proj_qk_group(n, w3, out_t, m):
                sl = slice(n * 512, (n + 1) * 512)
                p = ps_sm.tile([128, 512], F32, tag="sm", name="pp")
                for hf in range(2):
                    cs = slice(n * 512 + hf * 256, n * 512 + (hf + 1) * 256)
                    nmm = 12
                    i = 0
                    for wt, xt in QK_TERMS:
                        for c2 in range(KC // 2):
                            nc.tensor.matmul(
                                p[:, hf * 256 : (hf + 1) * 256],
                                lhsT=w3[:, wt, 2 * c2 : 2 * c2 + 2,
                                        m * 128 : (m + 1) * 128],
                                rhs=X3[:, xt, 2 * c2 : 2 * c2 + 2, cs],
                                start=(i == 0),
                                stop=(i == nmm - 1),
                                perf_mode=DR,
                                skip_group_check=True,
                            )
                            i += 1
                nc.vector.tensor_copy(out_t[:, m, sl], p[:])

            def proj_v_group(lt):
                p = ps_sm.tile([128, 512], F32, tag="sm", name="pv")[:, 0:256]
                nmm = 12
                i = 0
                for xt, wt in V_TERMS:
                    for c2 in range(KC // 2):
                        nc.tensor.matmul(
                            p[:],
                            lhsT=X3[:, xt, 2 * c2 : 2 * c2 + 2,
                                    lt * 128 : (lt + 1) * 128],
                            rhs=WV3[:, wt, 2 * c2 : 2 * c2 + 2, :],
                            start=(i == 0),
                            stop=(i == nmm - 1),
                            perf_mode=DR,
                            skip_group_check=True,
                        )
                        i += 1
                nc.vector.tensor_copy(
                    V[:, lt, :, 0:D], p[:].rearrange("p (h d) -> p h d", d=D)
                )

            def proj_groups(n):
                gs = []
                for w, out_t in ((WQ3, QT), (WK3, KT)):
                    for m in range(2):
                        gs.append(lambda w=w, o=out_t, m=m: proj_qk_group(n, w, o, m))
                for lt in range(4 * n, 4 * n + 4):
                    gs.append(lambda lt=lt: proj_v_group(lt))
                return gs

            def transp_group(j, AO, qt, c):
                tp = ps_sm.tile([128, 512], F32, tag="sm", name="tp")
                tp16 = tp[:].bitcast(F16)[:, 0:128]
                nc.tensor.transpose(
                    tp16[:], AO[:, qt, 2 * c : 2 * c + 2, :], EYE[:]
                )
                nc.vector.tensor_copy(
                    AOT[:, c, j * 512 + qt * 128 : j * 512 + (qt + 1) * 128], tp16[:]
                )

            def oproj_group(lt, ec, ysb, on_act=False):
                p = ps_sm.tile([128, 512], F32, tag="sm", name="py")
                for c in range(DH // 128):
                    nc.tensor.matmul(
                        p[:],
                        lhsT=AOT[:, c, lt * 128 : (lt + 1) * 128],
                        rhs=WO[:, c, ec * 512 : (ec + 1) * 512],
                        start=(c == 0),
                        stop=(c == DH // 128 - 1),
                    )
                cp = nc.scalar.copy if on_act else nc.vector.tensor_copy
                cp(ysb[:, ec * 512 : (ec + 1) * 512], p[:])
                nc.sync.dma_start(
                    y[lt * 128 : (lt + 1) * 128, ec * 512 : (ec + 1) * 512],
                    ysb[:, ec * 512 : (ec + 1) * 512],
                )

            def oproj_groups(j, on_act=False):
                gs = []
                for lt in range(4 * j, 4 * j + 4):
                    ysb = [None]

                    def g0(lt=lt, ysb=ysb):
                        ysb[0] = ysp.tile([128, 1024], F16, tag="ysb", name="ysb")
                        oproj_group(lt, 0, ysb[0], on_act)

                    def g1(lt=lt, ysb=ysb):
                        oproj_group(lt, 1, ysb[0], on_act)

                    gs += [g0, g1]
                return gs

            def transp_groups(j, AO):
                return [
                    (lambda qt=qt, c=c: transp_group(j, AO, qt, c))
                    for qt in range(4)
                    for c in range(2)
                ]

            def trail(j, AO, qt):
                """transpose + out-projection for q-subtile qt of chunk j:
                runnable as soon as both pairs' AV groups for qt are
                normalized."""
                transp_group(j, AO, qt, 0)
                transp_group(j, AO, qt, 1)
                lt = 4 * j + qt
                ysb = ysp.tile([128, 1024], F16, tag="ysb", name="ysb")
                oproj_group(lt, 0, ysb)
                oproj_group(lt, 1, ysb)

            # ---- attention for q-chunk j ----
            def attn(j, fillers=()):
                fillers = list(fillers)
                qsl = slice(j * 512, (j + 1) * 512)
                AO = aop.tile([128, 4, HL, D], F16, tag="AO", name="AO")
                nlk = 4 * j + 4
                nslots = 2 * nlk
                nfill = len(fillers)
                slot = [0]

                def pop_filler(diag):
                    # spread fillers evenly across the 2*nlk slots
                    want = (slot[0] + 1) * nfill // nslots if nslots else nfill
                    while fillers and (nfill - len(fillers)) < want:
                        fillers.pop(0)()
                    slot[0] += 1

                def av_group(ACC, pr, exlist, qt):
                    """AV accumulation groups for q-subtile qt (both heads of
                    the pair). Only one OPEN accumulation group per psum bank
                    at a time: each group's k-loop runs consecutively."""
                    last = 4 * j + qt
                    pidx = 2 * qt + pr  # completion order of pair groups
                    off = ACC_POFF[pidx if pidx < 6 else pidx - 6]
                    for h2 in range(2):
                        h = 2 * pr + h2
                        o = off + h2 * (D + 1)
                        for lk2 in range(last + 1):
                            nc.tensor.matmul(
                                ACC[:, o : o + D + 1],
                                lhsT=exlist[lk2][:, h2, qt * 128 : (qt + 1) * 128],
                                rhs=V[:, lk2, h, :],
                                start=(lk2 == 0),
                                stop=(lk2 == last),
                                skip_group_check=True,
                            )
                    # normalize both heads: AO[q, d] = acc[q, d] / acc[q, 64]
                    pv = ACC[:, off : off + 2 * (D + 1)].rearrange(
                        "p (h c) -> p h c", h=2
                    )
                    rec = rcp.tile([128, 2, 1], F32, tag="rec", name="rec")
                    nc.vector.reciprocal(rec[:], pv[:, :, D : D + 1])
                    nc.vector.tensor_mul(
                        AO[:, qt, 2 * pr : 2 * pr + 2, :],
                        pv[:, :, 0:D],
                        rec.to_broadcast([128, 2, D]),
                    )

                # both head pairs interleaved at the k-tile level: each
                # pair's exp gets a full iteration of slack before its S psum
                # buffer is needed again, hiding ACT latency from the PE.
                ACC = ps_acc.tile([128, 1024], F32, tag="acc", name="ACC")
                exl = ([], [])
                for lk in range(nlk):
                    m = lk - 4 * j  # >= 0 on diagonal tiles
                    c0 = 128 * m if m > 0 else 0  # first valid column
                    csl = slice(c0, 512)
                    for pr in range(HL // 2):
                        S = ps_s.tile([128, 2, 512], F32, tag="S", name="S")
                        ex = exs.tile([128, 2, 512], F16, tag="ex", name="ex")
                        for h2 in range(2):
                            hb = slice(h2 * 64, h2 * 64 + 64)
                            nc.tensor.matmul(
                                S[:, h2, csl],
                                lhsT=KT[hb, pr, lk * 128 : (lk + 1) * 128],
                                rhs=QT[hb, pr, qsl][:, csl],
                                start=True,
                                stop=True,
                            )
                        nc.scalar.activation(
                            ex[:, :, csl],
                            S[:, :, csl],
                            mybir.ActivationFunctionType.Exp,
                            scale=0.125 / 262144.0,
                        )
                        if m >= 0:
                            bvw = ex[:, :, c0 : c0 + 128]
                            nc.vector.tensor_mul(
                                bvw, bvw, CM[:, None, :].to_broadcast([128, 2, 128])
                            )
                        exl[pr].append(ex)
                    pop_filler(m >= 1)
                    if m >= 1:
                        av_group(ACC, 0, exl[0], m - 1)
                        av_group(ACC, 1, exl[1], m - 1)
                av_group(ACC, 0, exl[0], 3)
                av_group(ACC, 1, exl[1], 3)

                # leftover fillers
                for f in fillers:
                    f()
                return AO

            # ---- phases: proj staggered one chunk ahead of attention;
            # transp(j) runs inside attn(j+1), oproj(j) inside attn(j+2) so
            # PE filler work lands in the late, ACT-bound attention windows.
            for _rep in range(reps):
                if "proj" in phases:
                    for g in proj_groups(0):
                        g()
                aos = {}
                for st in range(NQ):
                    fillers = []
                    if "proj" in phases and st + 1 < NQ:
                        fillers += proj_groups(st + 1)
                    if "oproj" in phases and st - 1 in aos:
                        fillers += transp_groups(st - 1, aos[st - 1])
                    if "oproj" in phases and st >= 2:
                        fillers += oproj_groups(st - 2)
                    if "oproj" in phases and st == NQ - 1:
                        fillers += oproj_groups(st - 1)
                    if "attn" in phases:
                        aos[st] = attn(st, fillers)
                        if debug_out:
                            nc.sync.dma_start(dAO[st], aos[st][:])
                    else:
                        for f in fillers:
                            f()
                if "oproj" in phases and (NQ - 1) in aos:
                    tg = transp_groups(NQ - 1, aos[NQ - 1])
                    og = oproj_groups(NQ - 1, on_act=True)
                    for qt in range(4):
                        tg[2 * qt]()
                        tg[2 * qt + 1]()
                        og[2 * qt]()
                        og[2 * qt + 1]()
                if debug_out:
                    nc.sync.dma_start(dQT[:], QT[:])
                    nc.sync.dma_start(dKT[:], KT[:])
                    nc.sync.dma_start(dV[:], V[:])
                    nc.sync.dma_start(dAOT[:], AOT[:])
    nc.compile()
    return nc


def _x_pair(a, f8):
    """hi and lo*16 fp8e4m3 arrays of x for the compensated projection."""
    hi = a.astype(f8)
    lo16 = (16.0 * (a - hi.astype(np.float32))).astype(f8)
    return hi, lo16


def host_shard(net_in, W_qkv, W_out):
    """Full inputs -> list of 8 per-core input dicts."""
    import ml_dtypes

    f8 = ml_dtypes.float8_e4m3fn
    tri = (np.arange(128)[None, :] >= np.arange(128)[:, None]).astype(np.float16)
    eye = np.eye(128, dtype=np.float16)
    in_maps = []
    xtr = [None, None]
    for b in range(2):
        xtr[b] = _x_pair(np.ascontiguousarray(net_in[b].T), f8)
    for c in range(8):
        b, hg = divmod(c, 4)
        sl = slice(hg * DH, (hg + 1) * DH)
        xh, xl = xtr[b]

        def wtrip(wslice):
            # W' = 512 W; terms (hi, hi/16, lo); stacked [3, E, DH]
            wp = 512.0 * np.ascontiguousarray(wslice.T).astype(np.float32)
            hi = wp.astype(f8)
            hif = hi.astype(np.float32)
            his = (hif / 16.0).astype(f8)
            lo = (wp - hif).astype(f8)
            return np.stack([hi, his, lo])

        in_maps.append(
            {
                "xh": xh,
                "xl": xl,
                "wq3": wtrip(W_qkv[0 * E :][sl, :]),
                "wk3": wtrip(W_qkv[1 * E :][sl, :]),
                "wv3": wtrip(W_qkv[2 * E :][sl, :]),
                "woT": np.ascontiguousarray(W_out[:, sl].T).astype(np.float16),
                "tri": tri,
                "eye": eye,
            }
        )
    return in_maps


def host_unshard(results):
    """8 per-core result dicts -> full (B, L, E) output."""
    out = np.zeros((B, L, E), dtype=np.float32)
    for c in range(8):
        b = c // 4
        out[b] += np.asarray(results[c]["y"], dtype=np.float32)
    return out


_NC_CACHE = {}


def kernel(net_in, W_qkv, W_out):
    """Full inputs -> full (B, L, E) output, computed on 8 TRN2 NeuronCores."""
    net_in = np.ascontiguousarray(np.asarray(net_in, dtype=np.float32))
    W_qkv = np.ascontiguousarray(np.asarray(W_qkv, dtype=np.float32))
    W_out = np.ascontiguousarray(np.asarray(W_out, dtype=np.float32))

    if "nc" not in _NC_CACHE:
        _NC_CACHE["nc"] = build_kernel()
    nc = _NC_CACHE["nc"]

    in_maps = host_shard(net_in, W_qkv, W_out)
    from concourse import bass_utils

    res = bass_utils.run_bass_kernel_spmd(nc, in_maps, core_ids=list(range(8)))
    return host_unshard(res.results)
